# revision 1
# baseline (speedup 1.0000x reference)
"""Position-only MoE router kernel for Trainium2 (8 NeuronCores, SPMD).

Problem: x[8,2048,1024], tile_sigs[8,32], W[8,1024,1024], b[8,1024].
Routing idx[s] = argmax_t( pe[s] @ sign(tile_sigs[t]) ) depends only on the
position s, so it is computed on the host and baked into the kernel schedule
at build time. Each core processes one batch; tokens are permuted so rows
with the same expert are contiguous, then the kernel runs dense per-expert
matmuls out = lhsT.T @ rhs with stationary lhsT = X^T tile [d, 128s] and
moving rhs = W[e]^T tile [d, 512o]; bias is folded in as a K=1 matmul
(outer product ones[128] x b[e]).

Raw Bass (no Tile framework): explicit per-engine streams + semaphores.
  SP   : xt DMA, bias DMA, per-tile y stores
  ACT  : per-group W loads (double-buffered)
  PE   : matmuls
  DVE  : PSUM -> SBUF output copies
  POOL : ones memset
"""

import math
import os
import sys

import numpy as np

for _p in ("/opt/trn_rl_repo", "/opt/trn_rl_repo/concourse"):
    if _p not in sys.path and os.path.isdir(_p):
        sys.path.append(_p)

B, S, D, T, P = 8, 2048, 1024, 8, 32
NCORES = 8
KC = D // 128  # 8 contraction chunks
WS = 2  # W double-buffer slots
PS = 3  # PSUM accumulator slots
OS = 3  # output staging slots

# matmul dtype knob: "f32r" (1 cyc/row, tf32-ish) or "f32" (exact, 4 cyc/row)
MM_DT = os.environ.get("BASS_MOE_DT", "f32r")

LAST_RESULTS = None  # BassKernelResults of the most recent run (for profiling)
_CACHE = {}


def _routing_idx(tile_sigs: np.ndarray) -> np.ndarray:
    pos = np.arange(S, dtype=np.float32)[:, None]
    div = np.exp(
        np.arange(0, P, 2, dtype=np.float32) * (-math.log(10000.0) / P)
    ).astype(np.float32)
    ang = pos * div
    pe = np.zeros((S, P), np.float32)
    pe[:, 0::2] = np.sin(ang)
    pe[:, 1::2] = np.cos(ang)
    scores = pe @ np.sign(tile_sigs).astype(np.float32).T
    return np.argmax(scores, axis=-1)


def _plan(idx: np.ndarray):
    """Group positions by expert, pad each group to a multiple of 128."""
    POSP = []
    groups = []
    for e in range(T):
        pos_e = np.nonzero(idx == e)[0]
        if len(pos_e) == 0:
            continue
        pad = (-len(pos_e)) % 128
        padded = np.concatenate([pos_e, np.full(pad, pos_e[0], dtype=pos_e.dtype)])
        POSP.append(padded)
        groups.append((e, len(padded) // 128))
    POSP = np.concatenate(POSP)
    return POSP, groups


def _build_nc(npad: int, groups, mm_dt: str):
    import concourse.bass as bass
    import concourse.mybir as mybir

    f32 = mybir.dt.float32
    mmdt = mybir.dt.float32r if mm_dt == "f32r" else f32

    NT = npad // 128
    # tile t -> expert, and cumulative tile count at end of each group
    tile_expert = []
    t_end = []
    for e, ntiles in groups:
        tile_expert += [e] * ntiles
        t_end.append(len(tile_expert))
    NG = len(groups)

    nc = bass.Bass()
    # host layouts:
    #   xt [128,KC,npad]   xt[p,k,s] = x_perm[s, k*128+p]
    #   wt [T,128,KC,1024] wt[e,p,k,o] = W[e][o, k*128+p]
    #   bias [1, T*1024]
    xt_d = nc.dram_tensor("xt", [128, KC, npad], mmdt, kind="ExternalInput")
    wt_d = nc.dram_tensor("wt", [T, 128, KC, D], mmdt, kind="ExternalInput")
    bias_d = nc.dram_tensor("bias", [1, T * D], f32, kind="ExternalInput")
    y_d = nc.dram_tensor("y", [npad, D], f32, kind="ExternalOutput")

    from contextlib import ExitStack

    with ExitStack() as ctx:
        xt_sb = ctx.enter_context(nc.sbuf_tensor([128, KC, npad], mmdt))
        w_sb = ctx.enter_context(nc.sbuf_tensor([128, WS, KC, D], mmdt))
        bias_sb = ctx.enter_context(nc.sbuf_tensor([1, T * D], f32))
        ones_sb = ctx.enter_context(nc.sbuf_tensor([1, 128], f32))
        out_sb = ctx.enter_context(nc.sbuf_tensor([128, OS, D], f32))
        ps = ctx.enter_context(nc.psum_tensor([128, PS, D], f32))
        # one semaphore per DMA slot: increments on a given sem are strictly
        # serialized through the pipeline, so counts are race-free even
        # though independent DMAs can complete out of order.
        dma_xx = ctx.enter_context(nc.semaphore("dma_xx"))
        dma_xb = ctx.enter_context(nc.semaphore("dma_xb"))
        dma_w_s = [
            ctx.enter_context(nc.semaphore(f"dma_w{i}")) for i in range(WS)
        ]
        dma_y_s = [
            ctx.enter_context(nc.semaphore(f"dma_y{i}")) for i in range(OS)
        ]
        pe_t = ctx.enter_context(nc.semaphore("pe_t"))
        dve_c = ctx.enter_context(nc.semaphore("dve_c"))
        pool_m = ctx.enter_context(nc.semaphore("pool_m"))
        block = ctx.enter_context(nc.Block())

        y_count = [len(range(s, NT, OS)) for s in range(OS)]

        @block.gpsimd
        def _(eng):
            eng.memset(ones_sb[:], 1.0).then_inc(pool_m, 1)

        @block.sync
        def _(eng):
            eng.dma_start(xt_sb[:], xt_d[:]).then_inc(dma_xx, 16)
            eng.dma_start(bias_sb[:], bias_d[:]).then_inc(dma_xb, 16)
            for t in range(NT):
                eng.wait_ge(dve_c, t + 1)
                eng.dma_start(
                    y_d[t * 128 : (t + 1) * 128, :], out_sb[:, t % OS, :]
                ).then_inc(dma_y_s[t % OS], 16)
            for s in range(OS):
                eng.wait_ge(dma_y_s[s], 16 * y_count[s])

        @block.scalar
        def _(eng):
            for g, (e, ntiles) in enumerate(groups):
                if g >= WS:
                    eng.wait_ge(pe_t, t_end[g - WS])
                eng.dma_start(w_sb[:, g % WS, :, :], wt_d[e]).then_inc(
                    dma_w_s[g % WS], 16
                )

        @block.tensor
        def _(eng):
            eng.wait_ge(pool_m, 1)
            eng.wait_ge(dma_xx, 16)
            eng.wait_ge(dma_xb, 16)
            t = 0
            for g, (e, ntiles) in enumerate(groups):
                eng.wait_ge(dma_w_s[g % WS], 16 * (g // WS + 1))
                wslot = g % WS
                for _ in range(ntiles):
                    if t >= PS:
                        eng.wait_ge(dve_c, t - PS + 1)
                    pslot = t % PS
                    for k in range(KC):
                        lhsT = xt_sb[:, k, t * 128 : (t + 1) * 128]
                        for h in range(2):
                            eng.matmul(
                                ps[:, pslot, h * 512 : (h + 1) * 512],
                                lhsT,
                                w_sb[:, wslot, k, h * 512 : (h + 1) * 512],
                                start=(k == 0),
                                stop=False,
                            )
                    # bias: K=1 outer product ones[128] x b[e]; plain fp32
                    for h in range(2):
                        mm = eng.matmul(
                            ps[:, pslot, h * 512 : (h + 1) * 512],
                            ones_sb[0:1, :],
                            bias_sb[0:1, e * D + h * 512 : e * D + (h + 1) * 512],
                            start=False,
                            stop=True,
                        )
                    mm.then_inc(pe_t, 1)
                    t += 1

        @block.vector
        def _(eng):
            for t in range(NT):
                eng.wait_ge(pe_t, t + 1)
                if t >= OS:
                    eng.wait_ge(dma_y_s[t % OS], 16 * ((t - OS) // OS + 1))
                eng.tensor_copy(out_sb[:, t % OS, :], ps[:, t % PS, :]).then_inc(
                    dve_c, 1
                )

    return nc


def kernel(x, tile_sigs, W, b):
    global LAST_RESULTS
    from concourse.bass_utils import run_bass_kernel_spmd

    x = np.asarray(x, dtype=np.float32)
    tile_sigs = np.asarray(tile_sigs, dtype=np.float32)
    W = np.asarray(W, dtype=np.float32)
    b = np.asarray(b, dtype=np.float32)

    idx = _routing_idx(tile_sigs)
    POSP, groups = _plan(idx)
    npad = len(POSP)

    key = (npad, tuple(groups), MM_DT)
    if key in _CACHE:
        nc = _CACHE[key]
    else:
        nc = _build_nc(npad, groups, MM_DT)
        _CACHE[key] = nc

    # host-side shard prep
    wt = np.ascontiguousarray(
        W.transpose(0, 2, 1).reshape(T, KC, 128, D).transpose(0, 2, 1, 3)
    )
    bias = np.ascontiguousarray(b.reshape(1, T * D))
    in_maps = []
    for c in range(NCORES):
        xg = x[c][POSP]  # [npad, 1024]
        xt = np.ascontiguousarray(
            xg.T.reshape(KC, 128, npad).transpose(1, 0, 2)
        )  # [128, KC, npad]
        in_maps.append({"xt": xt, "wt": wt, "bias": bias})

    core_ids = list(range(NCORES))
    res = run_bass_kernel_spmd(nc, in_maps, core_ids)
    LAST_RESULTS = res

    out = np.empty((B, S, D), dtype=np.float32)
    for c in range(NCORES):
        yp = res.results[c]["y"]
        out[c][POSP] = yp
    return out



# revision 4
# speedup vs baseline: 2.4668x; 2.4668x over previous
"""Position-only MoE router kernel for Trainium2 (8 NeuronCores, SPMD).

Problem: x[8,2048,1024], tile_sigs[8,32], W[8,1024,1024], b[8,1024].
Routing idx[s] = argmax_t( pe[s] @ sign(tile_sigs[t]) ) depends only on the
position s, so it is computed on the host and baked into the schedule at
build time.

Strategy (token-parallel, expert-sorted):
  All B*S = 16384 tokens are grouped by expert and split into 8x17 tiles of
  128 tokens. Every core runs the IDENTICAL program (required: one NEFF,
  SPMD): 17 tiles in 4 groups of (13,2,1,1) tiles; each group uses one
  expert weight slot. Which expert each group is, and which tokens each
  tile holds, is per-core DATA packed by the host. x/W/y travel as bf16
  (fp32 PSUM accumulation), so per-core HBM traffic is ~16.5 MB vs the
  ~48 MB of a batch-parallel fp32 plan.

  Bias is applied without per-tile matmuls: once per group the PE computes
  ones[128] (x) b[e] into a dedicated PSUM region, DVE replicates it to
  SBUF, and the per-tile PSUM->SBUF drain becomes a fused add
  (scalar_tensor_tensor) on DVE.

Raw Bass (no Tile framework): explicit per-engine streams + semaphores.
  SP   : xt chunk DMAs, per-tile y stores
  ACT  : bias DMA, 4 per-group W loads
  PE   : per-group bias outer product + per-tile matmuls (8 K-chunks x 2)
  DVE  : per-group bias replication, per-tile fused add PSUM->SBUF
  POOL : ones memset
"""

import math
import os
import sys

import numpy as np

for _p in ("/opt/trn_rl_repo", "/opt/trn_rl_repo/concourse"):
    if _p not in sys.path and os.path.isdir(_p):
        sys.path.append(_p)

B, S, D, T, P = 8, 2048, 1024, 8, 32
NCORES = 8
KC = D // 128  # 8 contraction chunks
NT = 17  # tiles per core (8*17*128 = 17408 slots >= 16384 tokens)
SIZES = (13, 2, 1, 1)  # group sizes (tiles); one expert weight slot each
G = len(SIZES)
PS = 3  # PSUM accumulator slots
OS = 3  # output staging slots
XCHUNKS = [(0, 4), (4, 8), (8, 12), (12, 17)]  # xt DMA chunks (tiles)

LAST_RESULTS = None  # BassKernelResults of the most recent run (for profiling)
_CACHE = {}


def _routing_idx(tile_sigs: np.ndarray) -> np.ndarray:
    pos = np.arange(S, dtype=np.float32)[:, None]
    div = np.exp(
        np.arange(0, P, 2, dtype=np.float32) * (-math.log(10000.0) / P)
    ).astype(np.float32)
    ang = pos * div
    pe = np.zeros((S, P), np.float32)
    pe[:, 0::2] = np.sin(ang)
    pe[:, 1::2] = np.cos(ang)
    scores = pe @ np.sign(tile_sigs).astype(np.float32).T
    return np.argmax(scores, axis=-1)


def _plan(idx: np.ndarray):
    """Pack expert token lists into 8 cores x groups of SIZES tiles.

    Returns per-core list of (expert, ids) where ids is an int64 array of
    length size*128 with -1 marking padding rows.
    """
    # token ids (b*S + s) per expert, position-major
    ids_by_e = []
    for e in range(T):
        pos_e = np.nonzero(idx == e)[0]
        ids = (np.arange(B, dtype=np.int64)[:, None] * S + pos_e[None, :]).ravel()
        ids_by_e.append(ids)

    # part pool: SIZES[g] appears NCORES times
    from collections import Counter

    pool = Counter()
    for s in SIZES:
        pool[s] += NCORES
    sizes_desc = sorted(pool, reverse=True)

    parts_by_size = {s: [] for s in pool}
    order = sorted(range(T), key=lambda e: -len(ids_by_e[e]))
    for e in order:
        ids = ids_by_e[e]
        off = 0
        rem = len(ids)
        while rem > 0:
            # smallest size that covers the remainder with small padding,
            # else the largest size that fits fully
            cover = [s for s in sizes_desc if pool[s] > 0 and s * 128 >= rem]
            pick = None
            if cover and (min(cover) * 128 - rem) < 256:
                pick = min(cover)
            else:
                under = [s for s in sizes_desc if pool[s] > 0 and s * 128 <= rem]
                if under:
                    pick = max(under)
                elif cover:
                    pick = min(cover)
            if pick is None:
                raise RuntimeError("infeasible part decomposition")
            take = min(rem, pick * 128)
            chunk = np.full(pick * 128, -1, dtype=np.int64)
            chunk[:take] = ids[off : off + take]
            parts_by_size[pick].append((e, chunk))
            pool[pick] -= 1
            off += take
            rem -= take
    # leftover parts = pure padding (expert 0, all -1)
    for s in sizes_desc:
        while pool[s] > 0:
            parts_by_size[s].append((0, np.full(s * 128, -1, dtype=np.int64)))
            pool[s] -= 1

    # deal parts to cores: core c takes the next unused part of each size,
    # in SIZES order (repeated sizes take successive parts)
    taken = {s: 0 for s in parts_by_size}
    cores = []
    for c in range(NCORES):
        groups = []
        for s in SIZES:
            groups.append(parts_by_size[s][taken[s]])
            taken[s] += 1
        cores.append(groups)
    return cores


def _build_nc():
    import concourse.bass as bass
    import concourse.mybir as mybir

    f32 = mybir.dt.float32
    bf16 = mybir.dt.bfloat16

    nc = bass.Bass()
    # host layouts (per core):
    #   xt [128, NT, KC, 128]  xt[p,t,k,c] = x_tok[t*128+c, k*128+p]
    #   wt [G, 128, KC, D]     wt[g,p,k,o] = W[e_g][o, k*128+p]
    #   bias [1, G*D]          bias[0, g*D+o] = b[e_g][o]
    #   y [NT*128, D]          row-major tokens
    xt_d = nc.dram_tensor("xt", [128, NT, KC, 128], bf16, kind="ExternalInput")
    wt_d = nc.dram_tensor("wt", [G, 128, KC, D], bf16, kind="ExternalInput")
    bias_d = nc.dram_tensor("bias", [1, G * D], bf16, kind="ExternalInput")
    y_d = nc.dram_tensor("y", [NT * 128, D], bf16, kind="ExternalOutput")

    from contextlib import ExitStack

    # tile t -> group
    tile_group = []
    for g, sz in enumerate(SIZES):
        tile_group += [g] * sz

    def chunk_of(t):
        for ci, (a, bnd) in enumerate(XCHUNKS):
            if t < bnd:
                return ci
        raise AssertionError

    with ExitStack() as ctx:
        xt_sb = ctx.enter_context(nc.sbuf_tensor([128, NT, KC, 128], bf16))
        w_sb = ctx.enter_context(nc.sbuf_tensor([128, G, KC, D], bf16))
        bias_sb = ctx.enter_context(nc.sbuf_tensor([1, G * D], bf16))
        ones_sb = ctx.enter_context(nc.sbuf_tensor([1, 128], bf16))
        bias_rep = ctx.enter_context(nc.sbuf_tensor([128, 2, D], f32))
        out_sb = ctx.enter_context(nc.sbuf_tensor([128, OS, D], bf16))
        ps = ctx.enter_context(nc.psum_tensor([128, PS, D], f32))
        psb = ctx.enter_context(nc.psum_tensor([128, D], f32))

        dma_x = ctx.enter_context(nc.semaphore("dma_x"))
        dma_b = ctx.enter_context(nc.semaphore("dma_b"))
        dma_w = ctx.enter_context(nc.semaphore("dma_w"))
        dma_y_s = [
            ctx.enter_context(nc.semaphore(f"dma_y{i}")) for i in range(OS)
        ]
        pe_t = ctx.enter_context(nc.semaphore("pe_t"))
        pe_b = ctx.enter_context(nc.semaphore("pe_b"))
        bias_cp = ctx.enter_context(nc.semaphore("bias_cp"))
        dve_c = ctx.enter_context(nc.semaphore("dve_c"))
        pool_m = ctx.enter_context(nc.semaphore("pool_m"))
        block = ctx.enter_context(nc.Block())

        y_count = [len(range(s, NT, OS)) for s in range(OS)]

        @block.gpsimd
        def _(eng):
            eng.memset(ones_sb[:], 1.0).then_inc(pool_m, 1)

        @block.scalar
        def _(eng):
            eng.dma_start(bias_sb[:], bias_d[:]).then_inc(dma_b, 16)
            for g in range(G):
                eng.dma_start(w_sb[:, g, :, :], wt_d[g]).then_inc(dma_w, 16)

        @block.sync
        def _(eng):
            for a, bnd in XCHUNKS:
                eng.dma_start(
                    xt_sb[:, a:bnd, :, :], xt_d[:, a:bnd, :, :]
                ).then_inc(dma_x, 16)
            for t in range(NT):
                eng.wait_ge(dve_c, t + 1)
                eng.dma_start(
                    y_d[t * 128 : (t + 1) * 128, :], out_sb[:, t % OS, :]
                ).then_inc(dma_y_s[t % OS], 16)
            for s in range(OS):
                eng.wait_ge(dma_y_s[s], 16 * y_count[s])

        @block.tensor
        def _(eng):
            eng.wait_ge(pool_m, 1)
            eng.wait_ge(dma_b, 16)
            t = 0
            last_chunk = -1
            for g in range(G):
                if g >= 1:
                    eng.wait_ge(bias_cp, g)
                for h in range(2):
                    mm = eng.matmul(
                        psb[:, h * 512 : (h + 1) * 512],
                        ones_sb[0:1, :],
                        bias_sb[0:1, g * D + h * 512 : g * D + (h + 1) * 512],
                        start=True,
                        stop=True,
                    )
                mm.then_inc(pe_b, 1)
                eng.wait_ge(dma_w, 16 * (g + 1))
                for _i in range(SIZES[g]):
                    c = chunk_of(t)
                    if c > last_chunk:
                        eng.wait_ge(dma_x, 16 * (c + 1))
                        last_chunk = c
                    if t >= PS:
                        eng.wait_ge(dve_c, t - PS + 1)
                    for k in range(KC):
                        lhsT = xt_sb[:, t, k, :]
                        for h in range(2):
                            mm = eng.matmul(
                                ps[:, t % PS, h * 512 : (h + 1) * 512],
                                lhsT,
                                w_sb[:, g, k, h * 512 : (h + 1) * 512],
                                start=(k == 0),
                                stop=(k == KC - 1),
                            )
                    mm.then_inc(pe_t, 1)
                    t += 1

        @block.vector
        def _(eng):
            t = 0
            for g in range(G):
                eng.wait_ge(pe_b, g + 1)
                eng.tensor_copy(bias_rep[:, g % 2, :], psb[:, :]).then_inc(
                    bias_cp, 1
                )
                for _i in range(SIZES[g]):
                    eng.wait_ge(pe_t, t + 1)
                    if t >= OS:
                        eng.wait_ge(dma_y_s[t % OS], 16 * (t // OS))
                    eng.scalar_tensor_tensor(
                        out_sb[:, t % OS, :],
                        ps[:, t % PS, :],
                        0.0,
                        bias_rep[:, g % 2, :],
                        op0=mybir.AluOpType.add,
                        op1=mybir.AluOpType.add,
                    ).then_inc(dve_c, 1)
                    t += 1

    return nc


def kernel(x, tile_sigs, W, b):
    global LAST_RESULTS
    import ml_dtypes
    from concourse.bass_utils import run_bass_kernel_spmd

    bf16 = ml_dtypes.bfloat16

    x = np.asarray(x, dtype=np.float32)
    tile_sigs = np.asarray(tile_sigs, dtype=np.float32)
    W = np.asarray(W, dtype=np.float32)
    b = np.asarray(b, dtype=np.float32)

    idx = _routing_idx(tile_sigs)
    cores = _plan(idx)

    key = ("v2", NT, SIZES)
    if key in _CACHE:
        nc = _CACHE[key]
    else:
        nc = _build_nc()
        _CACHE[key] = nc

    # host-side shard prep (all bf16)
    xflat = np.ascontiguousarray(x.reshape(B * S, D)).astype(bf16)
    # wt_all[e][p,k,o] = W[e][o, k*128+p]
    wt_all = np.ascontiguousarray(
        W.transpose(0, 2, 1).reshape(T, KC, 128, D).transpose(0, 2, 1, 3)
    ).astype(bf16)
    b_bf = b.astype(bf16)

    in_maps = []
    ids_per_core = []
    for c in range(NCORES):
        groups = cores[c]
        ids = np.concatenate([g[1] for g in groups])  # [NT*128]
        ids_per_core.append(ids)
        safe = np.where(ids < 0, 0, ids)
        xg = xflat[safe]  # [NT*128, D] bf16
        xg[ids < 0] = 0
        xt = np.ascontiguousarray(
            xg.reshape(NT, 128, KC, 128).transpose(3, 0, 2, 1)
        )  # [128, NT, KC, 128]
        wt = np.ascontiguousarray(
            np.stack([wt_all[e] for e, _ in groups])
        )  # [G, 128, KC, D]
        bias = np.ascontiguousarray(
            np.stack([b_bf[e] for e, _ in groups]).reshape(1, G * D)
        )
        in_maps.append({"xt": xt, "wt": wt, "bias": bias})

    core_ids = list(range(NCORES))
    res = run_bass_kernel_spmd(nc, in_maps, core_ids)
    LAST_RESULTS = res

    out = np.empty((B * S, D), dtype=np.float32)
    for c in range(NCORES):
        yp = res.results[c]["y"]  # [NT*128, D] bf16
        ids = ids_per_core[c]
        valid = ids >= 0
        out[ids[valid]] = yp[valid].astype(np.float32)
    return out.reshape(B, S, D)
